# revision 37
# baseline (speedup 1.0000x reference)
"""Trainium2 Bass kernel for nn_InputLayer_57337813401914.

out[b, n, v] = sin(x[b, n] * freqs[v]), x: [64, 4096] f32,
freqs[v] = 10 ** (min(v, 127) / 127 * 4), v in [0, 256).

Sharding: batch dim (64) split across 8 NeuronCores; per core 32768 x
values. Rows 127..255 of the output are identical (freq index clamps at
127), so the device computes only the 128 distinct frequency rows as a
bf16 [128, 32768] tensor (tolerance is 2e-2; bf16 rounding costs ~1e-3)
and the host broadcasts row 127 into rows 128..255 while unsharding.
That cuts HBM writes from 33.5 MB to 8.4 MB per core, leaving the ACT
engine's Sin throughput (1 elem/lane/cycle @ 1.2 GHz => ~31.5 us incl.
per-instruction overhead) as the roofline instead of DMA.

Range reduction runs entirely inside the PE's sequential f32 PSUM
accumulation via one K=14 matmul per 512-col tile. Rows (stationary
f-side x moving x-side), in accumulation order:
    [fh fh fh fm fm fl] x [xh xm xl xh xm xh]   -> u = x*f/2pi (6 bf16
                                                   split products, every
                                                   term >= |u|*2^-24)
    [+MAGIC] x [1]                              -> fl(u+M) = M + rint(u)
    [-MAGIC] x [1]                              -> rint(u)   (Sterbenz)
    -(6 split rows again)                       -> rint(u) - u = -frac
PSUM then holds -frac(u) in [-0.5, 0.5] and ACT computes
Sin(-2pi * psum) = sin(2pi*u) = sin(x*f) straight out of PSUM into bf16
SBUF tiles. Verified on HW: ascending accumulation, rel err 1.4e-3.

Data movement (lessons from traced variants): x lives on 14 partitions
per half, so any one slice only engages 2 of the 16 SDMA engines --
wide/combined loads or HWDGE-ring loads just serialize behind ~0.6-1us
DMA_DIRECT2D issue costs on engines that must stay clean. So x streams
as 16 per-chunk slices on the gpsimd SWDGE ring, f14 + even-chunk
stores ride the sync HWDGE ring, odd-chunk stores ride the scalar ring
(ACT tolerates one ~0.6us trigger per 2 chunks without stretching the
1.96us sin cadence). Variants that tried to shave the ~12us head or
~5.9us tail (graded head chunks, combined f14+x loads, sync-ring x
loads, PE HAM warm-up matmuls, DVE polynomial-sin offload) all measured
0.7-13us SLOWER on HW: the PE is pinned at the gated 1.2 GHz clock here
(HAM never fires), leaving zero pipeline slack, and the Tile scheduler
serializes any extra PSUM consumer behind the sin. Measured best:
48.8us/core (baseline 104.7us).
"""
import numpy as np
from contextlib import ExitStack

import concourse.bacc as bacc
import concourse.tile as tile
from concourse import mybir
from concourse.alu_op_type import AluOpType as A
from concourse.bass_utils import run_bass_kernel_spmd

P = 128            # SBUF/PSUM partitions = number of distinct freqs
NX = 32768         # x values per core
V = 256            # total output rows (host replicates 128..255)
VLEAD = 128        # computed frequency rows
J = 2048           # x chunk per pipeline stage (4 PSUM banks)
NCHUNK = NX // J   # 16
MMN = 512          # moving free dim per matmul (one PSUM bank of fp32)
XROWS = 2          # x halves on partition bases 0 and 32 (matmul base rule)
XCOL = NX // XROWS
K14 = 14           # rows per matmul: 6 products, +M, -M, 6 negated products
NCORES = 8
B, N = 64, 4096
B_PER_CORE = B // NCORES

MAGIC = float(np.float32(1.5 * 2**23))
TWO_PI = float(np.float32(2.0 * np.pi))
ACT_SCALE = -TWO_PI  # PSUM holds -frac(u); sin(-2pi * -frac) = sin(2pi u)

# DISABLED (kept for reference): offloading the second half of a few
# chunks to DVE via a factored odd minimax polynomial
#   out = (s0p*b) * ((s0p*b)^2 - AP0) * ((s0p*b)^2 - AP1) ~= -sin(2pi*b)
# is numerically fine (adds ~4e-3 L2 on those halves; tolerance 2e-2) but
# measured SLOWER on HW: with the PE stuck at 1.2 GHz the pipeline has no
# slack, and the Tile scheduler serializes the DVE's PSUM read behind the
# half-sin, so every split injects a bubble that never heals.
SPLIT_CHUNKS = ()
JH = J // 2        # poly half-width
S0P = -2.187295984839723
AP0 = 1.19281746
AP1 = 2.34904642

_BUILT = None


def _freqs_lead() -> np.ndarray:
    """First 128 freqs, bit-matching the reference (jnp f32 ops)."""
    try:
        import jax.numpy as jnp

        f = 10.0 ** (jnp.arange(VLEAD, dtype=jnp.float32) / (VLEAD - 1) * 4.0)
        return np.asarray(f, dtype=np.float32)
    except Exception:
        y = np.arange(VLEAD, dtype=np.float32) / np.float32(VLEAD - 1)
        y = y * np.float32(4.0)
        return np.power(np.float32(10.0), y, dtype=np.float32)


def _freqs_over_2pi() -> np.ndarray:
    return (
        _freqs_lead().astype(np.float64) / (2.0 * np.pi)
    ).astype(np.float32)


def _build():
    nc = bacc.Bacc(
        "TRN2", target_bir_lowering=False, debug=False, num_devices=NCORES
    )
    f32 = mybir.dt.float32
    bf16 = mybir.dt.bfloat16
    x14_in = nc.dram_tensor(
        "x14", [XROWS * K14, XCOL], bf16, kind="ExternalInput"
    ).ap()
    f14_in = nc.dram_tensor(
        "f14", [XROWS * K14, P], bf16, kind="ExternalInput"
    ).ap()
    out_t = nc.dram_tensor("out", [VLEAD, NX], bf16, kind="ExternalOutput").ap()

    with tile.TileContext(nc) as tc:
        with ExitStack() as ctx:
            const = ctx.enter_context(tc.tile_pool(name="const", bufs=1))
            psum = ctx.enter_context(
                tc.tile_pool(name="psum", bufs=2, space="PSUM")
            )
            outp = ctx.enter_context(tc.tile_pool(name="outp", bufs=4))
            work = ctx.enter_context(tc.tile_pool(name="work", bufs=2))

            x14 = const.tile([32 + K14, XCOL], bf16, tag="x14")
            f14 = const.tile([32 + K14, P], bf16, tag="f14")
            nc.sync.dma_start(f14[0:K14, :], f14_in[0:K14, :])
            nc.sync.dma_start(
                f14[32 : 32 + K14, :], f14_in[K14 : 2 * K14, :]
            )
            # x lands on 14 partitions (2 SDMA engines); stream it as 16
            # per-chunk slices on the otherwise idle gpsimd SWDGE ring so
            # each slice arrives just ahead of its matmuls.
            for cc in range(NCHUNK):
                srow = 32 * (cc // 8)
                irow = K14 * (cc // 8)
                ccol = (cc % 8) * J
                nc.gpsimd.dma_start(
                    x14[srow : srow + K14, ccol : ccol + J],
                    x14_in[irow : irow + K14, ccol : ccol + J],
                )

            # preload the Sin spline tables (~1.3us) under the input DMAs
            warm = const.tile([1, 8], f32, tag="warm")
            nc.vector.memset(warm[:], 0.0)
            wsin = const.tile([1, 8], f32, tag="wsin")
            nc.scalar.activation(
                wsin[:], warm[:], mybir.ActivationFunctionType.Sin,
                bias=0.0, scale=1.0,
            )



            for c in range(NCHUNK):
                u_t = psum.tile([P, J], f32, tag="u")
                for m in range(J // MMN):
                    g = c * (J // MMN) + m
                    row = 32 * (g // (XCOL // MMN))
                    col = (g % (XCOL // MMN)) * MMN
                    nc.tensor.matmul(
                        u_t[:, m * MMN : (m + 1) * MMN],
                        f14[row : row + K14, :],
                        x14[row : row + K14, col : col + MMN],
                        start=True,
                        stop=True,
                    )
                s_t = outp.tile([P, J], bf16, tag="s")
                if c in SPLIT_CHUNKS:
                    nc.scalar.activation(
                        s_t[:, 0:JH], u_t[:, 0:JH],
                        mybir.ActivationFunctionType.Sin,
                        bias=0.0, scale=ACT_SCALE,
                    )
                    # DVE pass 1 drains PSUM cols JH:J concurrently with
                    # the sin above; passes 2-4 run later from SBUF.
                    b1 = work.tile([P, JH], f32, tag="b1")
                    nc.vector.tensor_scalar(
                        b1[:], u_t[:, JH:J], S0P, None, A.mult
                    )
                    q_t = work.tile([P, JH], f32, tag="q")
                    nc.vector.scalar_tensor_tensor(
                        q_t[:], b1[:], 1.0, b1[:], A.mult, A.mult
                    )
                    t1 = work.tile([P, JH], f32, tag="t1")
                    nc.vector.scalar_tensor_tensor(
                        t1[:], q_t[:], AP0, b1[:], A.subtract, A.mult
                    )
                    nc.vector.scalar_tensor_tensor(
                        s_t[:, JH:J], q_t[:], AP1, t1[:], A.subtract, A.mult
                    )
                else:
                    nc.scalar.activation(
                        s_t[:], u_t[:], mybir.ActivationFunctionType.Sin,
                        bias=0.0, scale=ACT_SCALE,
                    )
                ring = nc.sync if c % 2 == 0 else nc.scalar
                ring.dma_start(out_t[:, c * J : (c + 1) * J], s_t[:])

    nc.compile()
    return nc


def _split3(a: np.ndarray):
    """Exact-ish 3-way bf16 split: h + m + l == a to within ~2^-25 rel."""
    import ml_dtypes

    bf = ml_dtypes.bfloat16
    h = a.astype(bf)
    m = (a - h.astype(np.float32)).astype(bf)
    l = (a - h.astype(np.float32) - m.astype(np.float32)).astype(bf)
    return h, m, l


def _in_maps(x: np.ndarray):
    """Per-core input dict with the K=14 magic-reduction row stacks."""
    import ml_dtypes

    bf = ml_dtypes.bfloat16
    fp = _freqs_over_2pi()
    fh, fm, fl = _split3(fp)
    mrow = np.full(P, MAGIC, dtype=bf)
    f_block = np.stack(
        [fh, fh, fh, fm, fm, fl, mrow, -mrow, -fh, -fh, -fh, -fm, -fm, -fl]
    )  # [K14, P] bf16
    f14 = np.ascontiguousarray(np.tile(f_block, (XROWS, 1)).astype(bf))

    ones = np.ones(XCOL, dtype=bf)
    in_maps = []
    for c in range(NCORES):
        xs = x[c * B_PER_CORE : (c + 1) * B_PER_CORE].reshape(XROWS, XCOL)
        xh, xm, xl = _split3(xs)
        rows = []
        for i in range(XROWS):
            rows += [xh[i], xm[i], xl[i], xh[i], xm[i], xh[i], ones, ones,
                     xh[i], xm[i], xl[i], xh[i], xm[i], xh[i]]
        x14 = np.ascontiguousarray(np.stack(rows))
        in_maps.append({"x14": x14, "f14": f14})
    return in_maps


def kernel(x, vector_size):
    global _BUILT
    x = np.asarray(x, dtype=np.float32)
    assert x.shape == (B, N), x.shape
    assert int(vector_size) == V, vector_size

    if _BUILT is None:
        _BUILT = _build()
    nc = _BUILT

    res = run_bass_kernel_spmd(nc, _in_maps(x), list(range(NCORES)))

    out = np.empty((B, N, V), dtype=np.float32)
    for c in range(NCORES):
        oc = np.asarray(res.results[c]["out"]).astype(np.float32)
        blk = np.ascontiguousarray(oc.T).reshape(B_PER_CORE, N, VLEAD)
        out[c * B_PER_CORE : (c + 1) * B_PER_CORE, :, :VLEAD] = blk
        out[c * B_PER_CORE : (c + 1) * B_PER_CORE, :, VLEAD:] = blk[
            :, :, VLEAD - 1 :
        ]
    return out


# revision 40
# speedup vs baseline: 1.0450x; 1.0450x over previous
"""Trainium2 Bass kernel for nn_InputLayer_57337813401914.

out[b, n, v] = sin(x[b, n] * freqs[v]), x: [64, 4096] f32,
freqs[v] = 10 ** (min(v, 127) / 127 * 4), v in [0, 256).

Sharding: batch dim (64) split across 8 NeuronCores; per core 32768 x
values. Rows 127..255 of the output are identical (freq index clamps at
127), so the device computes only the 128 distinct frequency rows as a
bf16 [128, 32768] tensor (tolerance is 2e-2; bf16 rounding costs ~1e-3)
and the host broadcasts row 127 into rows 128..255 while unsharding.
That cuts HBM writes from 33.5 MB to 8.4 MB per core, leaving the ACT
engine's Sin throughput (1 elem/lane/cycle @ 1.2 GHz => ~31.5 us incl.
per-instruction overhead) as the roofline instead of DMA.

Range reduction runs entirely inside the PE's sequential f32 PSUM
accumulation via one K=14 matmul per 512-col tile. Rows (stationary
f-side x moving x-side), in accumulation order:
    [fh fh fh fm fm fl] x [xh xm xl xh xm xh]   -> u = x*f/2pi (6 bf16
                                                   split products, every
                                                   term >= |u|*2^-24)
    [+MAGIC] x [1]                              -> fl(u+M) = M + rint(u)
    [-MAGIC] x [1]                              -> rint(u)   (Sterbenz)
    -(6 split rows again)                       -> rint(u) - u = -frac
PSUM then holds -frac(u) in [-0.5, 0.5] and ACT computes
Sin(-2pi * psum) = sin(2pi*u) = sin(x*f) straight out of PSUM into bf16
SBUF tiles. Verified on HW: ascending accumulation, rel err 1.4e-3.

Data movement (lessons from traced variants): x lives on 14 partitions
per half, so any one slice only engages 2 of the 16 SDMA engines --
wide/combined loads or HWDGE-ring loads just serialize behind ~0.6-1us
DMA_DIRECT2D issue costs on engines that must stay clean. So x streams
as 16 per-chunk slices on the gpsimd SWDGE ring, f14 + even-chunk
stores ride the sync HWDGE ring, odd-chunk stores ride the scalar ring
(ACT tolerates one ~0.6us trigger per 2 chunks without stretching the
1.96us sin cadence). Variants that tried to shave the ~12us head or
~5.9us tail (graded head chunks, combined f14+x loads, sync-ring x
loads, PE HAM warm-up matmuls, DVE polynomial-sin offload) all measured
0.7-13us SLOWER on HW: the PE is pinned at the gated 1.2 GHz clock here
(HAM never fires), leaving zero pipeline slack, and the Tile scheduler
serializes any extra PSUM consumer behind the sin. Measured best:
48.8us/core (baseline 104.7us).
"""
import numpy as np
from contextlib import ExitStack

import concourse.bacc as bacc
import concourse.tile as tile
from concourse import mybir
from concourse.alu_op_type import AluOpType as A
from concourse.bass_utils import run_bass_kernel_spmd

P = 128            # SBUF/PSUM partitions = number of distinct freqs
NX = 32768         # x values per core
V = 256            # total output rows (host replicates 128..255)
VLEAD = 128        # computed frequency rows
J = 2048           # x chunk per pipeline stage (4 PSUM banks)
NCHUNK = NX // J   # 16
MMN = 512          # moving free dim per matmul (one PSUM bank of fp32)
XROWS = 2          # x halves on partition bases 0 and 32 (matmul base rule)
XCOL = NX // XROWS
K14 = 14           # rows per matmul: 6 products, +M, -M, 6 negated products
NCORES = 8
B, N = 64, 4096
B_PER_CORE = B // NCORES

MAGIC = float(np.float32(1.5 * 2**23))
TWO_PI = float(np.float32(2.0 * np.pi))
ACT_SCALE = -TWO_PI  # PSUM holds -frac(u); sin(-2pi * -frac) = sin(2pi u)

# DISABLED (kept for reference): offloading the second half of a few
# chunks to DVE via a factored odd minimax polynomial
#   out = (s0p*b) * ((s0p*b)^2 - AP0) * ((s0p*b)^2 - AP1) ~= -sin(2pi*b)
# is numerically fine (adds ~4e-3 L2 on those halves; tolerance 2e-2) but
# measured SLOWER on HW: with the PE stuck at 1.2 GHz the pipeline has no
# slack, and the Tile scheduler serializes the DVE's PSUM read behind the
# half-sin, so every split injects a bubble that never heals.
SPLIT_CHUNKS = ()
JH = J // 2        # poly half-width
S0P = -2.187295984839723
AP0 = 1.19281746
AP1 = 2.34904642

_BUILT = None


def _freqs_lead() -> np.ndarray:
    """First 128 freqs, bit-matching the reference (jnp f32 ops)."""
    try:
        import jax.numpy as jnp

        f = 10.0 ** (jnp.arange(VLEAD, dtype=jnp.float32) / (VLEAD - 1) * 4.0)
        return np.asarray(f, dtype=np.float32)
    except Exception:
        y = np.arange(VLEAD, dtype=np.float32) / np.float32(VLEAD - 1)
        y = y * np.float32(4.0)
        return np.power(np.float32(10.0), y, dtype=np.float32)


def _freqs_over_2pi() -> np.ndarray:
    return (
        _freqs_lead().astype(np.float64) / (2.0 * np.pi)
    ).astype(np.float32)


def _build():
    nc = bacc.Bacc(
        "TRN2", target_bir_lowering=False, debug=False, num_devices=NCORES
    )
    f32 = mybir.dt.float32
    bf16 = mybir.dt.bfloat16
    x14_in = nc.dram_tensor(
        "x14", [XROWS * K14, XCOL], bf16, kind="ExternalInput"
    ).ap()
    f14_in = nc.dram_tensor(
        "f14", [XROWS * K14, P], bf16, kind="ExternalInput"
    ).ap()
    out_t = nc.dram_tensor("out", [VLEAD, NX], bf16, kind="ExternalOutput").ap()

    with tile.TileContext(nc) as tc:
        with ExitStack() as ctx:
            const = ctx.enter_context(tc.tile_pool(name="const", bufs=1))
            psum = ctx.enter_context(
                tc.tile_pool(name="psum", bufs=2, space="PSUM")
            )
            outp = ctx.enter_context(tc.tile_pool(name="outp", bufs=4))
            work = ctx.enter_context(tc.tile_pool(name="work", bufs=2))

            x14 = const.tile([32 + K14, XCOL], bf16, tag="x14")
            f14 = const.tile([32 + K14, P], bf16, tag="f14")
            nc.sync.dma_start(f14[0:K14, :], f14_in[0:K14, :])
            nc.sync.dma_start(
                f14[32 : 32 + K14, :], f14_in[K14 : 2 * K14, :]
            )
            # chunk geometry: (half, local col, cols). 15 full 2048-chunks
            # plus two 1024 tail chunks so the final store is only 256KB
            # (the post-last-sin drain is on the critical path).
            geo = [(0, cc * J, J) for cc in range(8)]
            geo += [(1, cc * J, J) for cc in range(7)]
            geo += [(1, 7 * J, J // 2), (1, 7 * J + J // 2, J // 2)]
            assert sum(g[2] for g in geo) == NX

            # x lands on 14 partitions (2 SDMA engines); stream it as
            # per-chunk slices on the otherwise idle gpsimd SWDGE ring so
            # each slice arrives just ahead of its matmuls. Chunk 0 comes
            # as two 1024-col slices: matmuls 0-1 start while the second
            # half is still in flight, pulling the first Sin ~0.6us earlier.
            for cc, (half, l0, jlen) in enumerate(geo):
                srow = 32 * half
                irow = K14 * half
                subs = (
                    [(l0, J // 2), (l0 + J // 2, J // 2)]
                    if cc == 0
                    else [(l0, jlen)]
                )
                for s0, slen in subs:
                    nc.gpsimd.dma_start(
                        x14[srow : srow + K14, s0 : s0 + slen],
                        x14_in[irow : irow + K14, s0 : s0 + slen],
                    )

            # preload the Sin spline tables (~1.3us) under the input DMAs
            warm = const.tile([1, 8], f32, tag="warm")
            nc.vector.memset(warm[:], 0.0)
            wsin = const.tile([1, 8], f32, tag="wsin")
            nc.scalar.activation(
                wsin[:], warm[:], mybir.ActivationFunctionType.Sin,
                bias=0.0, scale=1.0,
            )



            for c, (half, l0, jlen) in enumerate(geo):
                u_t = psum.tile([P, J], f32, tag="u")
                row = 32 * half
                for m in range(jlen // MMN):
                    col = l0 + m * MMN
                    nc.tensor.matmul(
                        u_t[:, m * MMN : (m + 1) * MMN],
                        f14[row : row + K14, :],
                        x14[row : row + K14, col : col + MMN],
                        start=True,
                        stop=True,
                    )
                gcol = half * XCOL + l0
                s_t = outp.tile([P, J], bf16, tag="s")
                if c in SPLIT_CHUNKS:
                    nc.scalar.activation(
                        s_t[:, 0:JH], u_t[:, 0:JH],
                        mybir.ActivationFunctionType.Sin,
                        bias=0.0, scale=ACT_SCALE,
                    )
                    # DVE pass 1 drains PSUM cols JH:J concurrently with
                    # the sin above; passes 2-4 run later from SBUF.
                    b1 = work.tile([P, JH], f32, tag="b1")
                    nc.vector.tensor_scalar(
                        b1[:], u_t[:, JH:J], S0P, None, A.mult
                    )
                    q_t = work.tile([P, JH], f32, tag="q")
                    nc.vector.scalar_tensor_tensor(
                        q_t[:], b1[:], 1.0, b1[:], A.mult, A.mult
                    )
                    t1 = work.tile([P, JH], f32, tag="t1")
                    nc.vector.scalar_tensor_tensor(
                        t1[:], q_t[:], AP0, b1[:], A.subtract, A.mult
                    )
                    nc.vector.scalar_tensor_tensor(
                        s_t[:, JH:J], q_t[:], AP1, t1[:], A.subtract, A.mult
                    )
                else:
                    nc.scalar.activation(
                        s_t[:, 0:jlen], u_t[:, 0:jlen],
                        mybir.ActivationFunctionType.Sin,
                        bias=0.0, scale=ACT_SCALE,
                    )
                ring = nc.sync if c % 2 == 0 else nc.scalar
                ring.dma_start(
                    out_t[:, gcol : gcol + jlen], s_t[:, 0:jlen]
                )

    nc.compile()
    return nc


def _split3(a: np.ndarray):
    """Exact-ish 3-way bf16 split: h + m + l == a to within ~2^-25 rel."""
    import ml_dtypes

    bf = ml_dtypes.bfloat16
    h = a.astype(bf)
    m = (a - h.astype(np.float32)).astype(bf)
    l = (a - h.astype(np.float32) - m.astype(np.float32)).astype(bf)
    return h, m, l


def _in_maps(x: np.ndarray):
    """Per-core input dict with the K=14 magic-reduction row stacks."""
    import ml_dtypes

    bf = ml_dtypes.bfloat16
    fp = _freqs_over_2pi()
    fh, fm, fl = _split3(fp)
    mrow = np.full(P, MAGIC, dtype=bf)
    f_block = np.stack(
        [fh, fh, fh, fm, fm, fl, mrow, -mrow, -fh, -fh, -fh, -fm, -fm, -fl]
    )  # [K14, P] bf16
    f14 = np.ascontiguousarray(np.tile(f_block, (XROWS, 1)).astype(bf))

    ones = np.ones(XCOL, dtype=bf)
    in_maps = []
    for c in range(NCORES):
        xs = x[c * B_PER_CORE : (c + 1) * B_PER_CORE].reshape(XROWS, XCOL)
        xh, xm, xl = _split3(xs)
        rows = []
        for i in range(XROWS):
            rows += [xh[i], xm[i], xl[i], xh[i], xm[i], xh[i], ones, ones,
                     xh[i], xm[i], xl[i], xh[i], xm[i], xh[i]]
        x14 = np.ascontiguousarray(np.stack(rows))
        in_maps.append({"x14": x14, "f14": f14})
    return in_maps


def kernel(x, vector_size):
    global _BUILT
    x = np.asarray(x, dtype=np.float32)
    assert x.shape == (B, N), x.shape
    assert int(vector_size) == V, vector_size

    if _BUILT is None:
        _BUILT = _build()
    nc = _BUILT

    res = run_bass_kernel_spmd(nc, _in_maps(x), list(range(NCORES)))

    out = np.empty((B, N, V), dtype=np.float32)
    for c in range(NCORES):
        oc = np.asarray(res.results[c]["out"]).astype(np.float32)
        blk = np.ascontiguousarray(oc.T).reshape(B_PER_CORE, N, VLEAD)
        out[c * B_PER_CORE : (c + 1) * B_PER_CORE, :, :VLEAD] = blk
        out[c * B_PER_CORE : (c + 1) * B_PER_CORE, :, VLEAD:] = blk[
            :, :, VLEAD - 1 :
        ]
    return out
